# revision 7
# baseline (speedup 1.0000x reference)
"""Trainium2 Bass kernel for the Adapter module (nn_Adapter_63436666962301).

Data-parallel over batch: B=32 split as 4 batches per NeuronCore x 8 cores.
Math per batch (reference):
  att_y2t = softmax(latent @ y^T, axis=j)           [T, Sy]
  tokens  = latent + att_y2t @ y                    [T, D]
  att_t2x = softmax(x @ tokens^T, axis=t)           [Sx, T]
  x_new   = x + gate * (att_t2x @ tokens)
  out     = relu(x_new @ W_down^T) @ W_up^T

On-chip we fold the gated attention into the down projection:
  z_preT[e, s] = sum_d WdT[d, e]^T xT[d, s] + (gate * tokens@Wd^T)^T[e, t] attT[t, s]
which is exact by distributivity, so the big x tensor streams through the
TensorEngine only twice (t2x logits + down-proj).  T=2 softmax over tokens is
computed as sigmoid(l0 - l1) / sigmoid(l1 - l0) with the difference produced by
a tiny matmul against [[1,-1],[-1,1]].

Compute dtype: bf16 operands, fp32 PSUM accumulation (rel err ~2-3e-3).
"""

import os
import sys
import types

import numpy as np
import ml_dtypes

BF16 = ml_dtypes.bfloat16

# ---- problem constants (hardcoded; kernel.py must be self-contained) ----
N_CORES = 8
B_GLOBAL = 32
B = B_GLOBAL // N_CORES  # 4 batches per core
SX = 2048
SY = 512
D = 1024
T = 2
E = 128   # bottleneck dim (D // 8)
O = 1024  # output dim
S = B * SX              # 8192 rows of x per core
CH = 512                # s-chunk width
NCH = S // CH           # 16 chunks
KD = D // 128           # 8 contraction tiles
CH_PER_B = SX // CH     # 4 chunks per batch
JT = SY // 128          # 4 j-tiles of y


def _install_axon_ntff_hook():
    """Register the NTFF profiling hook that this image's antenv lacks."""
    try:
        from antenv.axon_hooks import get_axon_ntff_profile_hook  # noqa: F401
        return
    except ImportError:
        pass
    try:
        import antenv
        from trn_agent_boot.trn_boot import _ntff_profile_via_ctypes
        hook = _ntff_profile_via_ctypes("/opt/axon/libaxon_pjrt.so")
    except Exception:
        return
    mod = types.ModuleType("antenv.axon_hooks")
    mod._hook = hook
    mod.get_axon_ntff_profile_hook = lambda: mod._hook

    def _set(h):
        mod._hook = h

    mod.set_axon_ntff_profile_hook = _set
    sys.modules["antenv.axon_hooks"] = mod
    antenv.axon_hooks = mod


_NC_CACHE = {}
LAST_RESULT = None  # test.py reads exec_time_ns from here


def _build():
    import concourse.bass as bass
    import concourse.tile as tile
    from concourse import bacc, mybir

    f32 = mybir.dt.float32
    bf16 = mybir.dt.bfloat16

    nc = bacc.Bacc("TRN2", target_bir_lowering=False, debug=False)

    # ---- DRAM parameters (per-core shard shapes) ----
    xT_d = nc.dram_tensor("xT", [KD, 128, S], bf16, kind="ExternalInput").ap()
    yT_d = nc.dram_tensor("yT", [B, KD, 128, SY], bf16, kind="ExternalInput").ap()
    yn_d = nc.dram_tensor("ynat", [B, JT, 128, D], bf16, kind="ExternalInput").ap()
    latT_d = nc.dram_tensor("latT", [KD, 128, T], bf16, kind="ExternalInput").ap()
    lat_d = nc.dram_tensor("latent", [T, D], f32, kind="ExternalInput").ap()
    wdT_d = nc.dram_tensor("wdT", [KD, 128, E], bf16, kind="ExternalInput").ap()
    wuT_d = nc.dram_tensor("wuT", [E, O], bf16, kind="ExternalInput").ap()
    gate_d = nc.dram_tensor("gate128", [128, 1], f32, kind="ExternalInput").ap()
    id2_d = nc.dram_tensor("id2", [T, T], bf16, kind="ExternalInput").ap()
    out_d = nc.dram_tensor("out", [S, O], bf16, kind="ExternalOutput").ap()

    with tile.TileContext(nc) as tc:
        with (
            tc.tile_pool(name="const", bufs=1) as const,
            tc.tile_pool(name="ypool", bufs=2) as ypool,
            tc.tile_pool(name="xpool", bufs=3) as xpool,
            tc.tile_pool(name="work", bufs=2) as work,
            tc.tile_pool(name="tokw", bufs=1) as tokw,
            tc.tile_pool(name="psum", bufs=1, space="PSUM") as psum,
        ):
            # ---- constants ----
            wdT_sb = const.tile([128, KD, E], bf16)
            nc.sync.dma_start(out=wdT_sb[:], in_=wdT_d.rearrange("k p e -> p k e"))
            wuT_sb = const.tile([E, O], bf16)
            nc.sync.dma_start(out=wuT_sb[:], in_=wuT_d[:])
            latT_sb = const.tile([128, KD, T], bf16)
            nc.sync.dma_start(out=latT_sb[:], in_=latT_d.rearrange("k p t -> p k t"))
            lat_sb = const.tile([T, D], f32)
            nc.sync.dma_start(out=lat_sb[:], in_=lat_d[:])
            gate_sb = const.tile([128, 1], f32)
            nc.sync.dma_start(out=gate_sb[:], in_=gate_d[:])
            id2_sb = const.tile([T, T], bf16)
            nc.sync.dma_start(out=id2_sb[:], in_=id2_d[:])

            # per-batch token state (lives across the batch's 4 chunks)
            tokT_sb = tokw.tile([128, B, KD, T], bf16)   # tokens^T, bf16
            tokDT_sb = tokw.tile([128, B, KD, T], bf16)  # [t0-t1, t1-t0] columns
            gtd_sb = tokw.tile([T, B, E], bf16)          # gate * (tokens @ Wd^T)

            def phase_a(b):
                """Per-batch: y2t attention -> tokens -> tokensT, gate*tokens_down."""
                yT_sb = ypool.tile([128, KD, SY], bf16, tag="yT")
                nc.sync.dma_start(out=yT_sb[:], in_=yT_d[b].rearrange("k p j -> p k j"))
                yn_sb = ypool.tile([128, JT, D], bf16, tag="ynat")
                nc.sync.dma_start(out=yn_sb[:], in_=yn_d[b].rearrange("j p d -> p j d"))

                # scores[t, j] = latent @ y^T (contraction over d)
                ps_sc = psum.tile([T, SY], f32, tag="tokp")
                for kd in range(KD):
                    nc.tensor.matmul(
                        ps_sc[:], latT_sb[:, kd, :], yT_sb[:, kd, :],
                        start=(kd == 0), stop=(kd == KD - 1),
                    )
                # softmax over j (free dim)
                negmx = work.tile([T, 1], f32, tag="small")
                nc.vector.tensor_reduce(
                    negmx[:], ps_sc[:], mybir.AxisListType.X, mybir.AluOpType.max,
                    negate=True,
                )
                e_sb = work.tile([T, SY], f32, tag="esb")
                nc.scalar.activation(
                    e_sb[:], ps_sc[:], mybir.ActivationFunctionType.Exp,
                    bias=negmx[:], scale=1.0,
                )
                ssum = work.tile([T, 1], f32, tag="small")
                nc.vector.tensor_reduce(
                    ssum[:], e_sb[:], mybir.AxisListType.X, mybir.AluOpType.add,
                )
                rinv = work.tile([T, 1], f32, tag="small")
                nc.vector.reciprocal(rinv[:], ssum[:])
                att_bf = work.tile([T, SY], bf16, tag="atty")
                nc.vector.tensor_scalar_mul(att_bf[:], e_sb[:], rinv[:])

                # att^T via PE transpose (4 j-tiles)
                attT_sb = work.tile([128, JT, T], bf16, tag="attT")
                for jt in range(JT):
                    ps_at = psum.tile([128, T], bf16, tag="tokp")
                    nc.tensor.transpose(
                        ps_at[:], att_bf[:, jt * 128:(jt + 1) * 128], id2_sb[:]
                    )
                    nc.vector.tensor_copy(attT_sb[:, jt, :], ps_at[:])

                # tokens[t, d] = latent + att @ y  (contraction over j), halves of d
                tok_bf = work.tile([T, D], bf16, tag="tok")
                for dh in range(2):
                    ps_tok = psum.tile([T, 512], f32, tag="tokp")
                    for jt in range(JT):
                        nc.tensor.matmul(
                            ps_tok[:], attT_sb[:, jt, :],
                            yn_sb[:, jt, dh * 512:(dh + 1) * 512],
                            start=(jt == 0), stop=(jt == JT - 1),
                        )
                    nc.vector.tensor_add(
                        tok_bf[:, dh * 512:(dh + 1) * 512], ps_tok[:],
                        lat_sb[:, dh * 512:(dh + 1) * 512],
                    )

                # tokens^T via PE transpose (8 d-tiles)
                for kd in range(KD):
                    ps_tt = psum.tile([128, T], bf16, tag="tokp")
                    nc.tensor.transpose(
                        ps_tt[:], tok_bf[:, kd * 128:(kd + 1) * 128], id2_sb[:]
                    )
                    nc.vector.tensor_copy(tokT_sb[:, b, kd, :], ps_tt[:])
                    # difference columns for the T=2 softmax-as-sigmoid
                    nc.vector.tensor_sub(
                        tokDT_sb[:, b, kd, 0:1],
                        tokT_sb[:, b, kd, 0:1], tokT_sb[:, b, kd, 1:2],
                    )
                    nc.vector.tensor_sub(
                        tokDT_sb[:, b, kd, 1:2],
                        tokT_sb[:, b, kd, 1:2], tokT_sb[:, b, kd, 0:1],
                    )

                # tokens_down[t, e] = tokens @ Wd^T, then scale by gate
                ps_td = psum.tile([T, E], f32, tag="tokp")
                for kd in range(KD):
                    nc.tensor.matmul(
                        ps_td[:], tokT_sb[:, b, kd, :], wdT_sb[:, kd, :],
                        start=(kd == 0), stop=(kd == KD - 1),
                    )
                nc.vector.tensor_scalar_mul(gtd_sb[:, b, :], ps_td[:], gate_sb[0:T, :])

            def phase_b(c):
                """Per s-chunk of 512 rows: t2x attention + down + up projection."""
                b = c // CH_PER_B
                c0 = c * CH
                x_sb = xpool.tile([128, KD, CH], bf16, tag="xT")
                nc.sync.dma_start(
                    out=x_sb[:], in_=xT_d[:, :, c0:c0 + CH].rearrange("k p s -> p k s")
                )

                # logit diffs [l0-l1; l1-l0] directly (contraction over d)
                ps_dd = psum.tile([T, CH], f32, tag="dd", bufs=2)
                for kd in range(KD):
                    nc.tensor.matmul(
                        ps_dd[:], tokDT_sb[:, b, kd, :], x_sb[:, kd, :],
                        start=(kd == 0), stop=(kd == KD - 1),
                    )
                attx_bf = work.tile([T, CH], bf16, tag="attx")
                nc.scalar.activation(
                    attx_bf[:], ps_dd[:], mybir.ActivationFunctionType.Sigmoid,
                )

                # z^T[e, s] = Wd @ x_new^T  (+ gated attention term), then relu
                ps_z = psum.tile([E, CH], f32, tag="z", bufs=2)
                for kd in range(KD):
                    nc.tensor.matmul(
                        ps_z[:], wdT_sb[:, kd, :], x_sb[:, kd, :],
                        start=(kd == 0), stop=False,
                    )
                nc.tensor.matmul(
                    ps_z[:], gtd_sb[:, b, :], attx_bf[:],
                    start=False, stop=True,
                )
                z_bf = work.tile([E, CH], bf16, tag="z_bf")
                nc.scalar.activation(
                    z_bf[:], ps_z[:], mybir.ActivationFunctionType.Relu,
                )

                # out[s, o] = z @ Wu^T  (contraction over e=128, single K tile)
                for st in range(4):
                    o_bf = work.tile([128, O], bf16, tag="obf")
                    for oh in range(2):
                        ps_o = psum.tile([128, 512], f32, tag="o", bufs=3)
                        nc.tensor.matmul(
                            ps_o[:], z_bf[:, st * 128:(st + 1) * 128],
                            wuT_sb[:, oh * 512:(oh + 1) * 512],
                        )
                        if oh == 0:
                            nc.vector.tensor_copy(o_bf[:, 0:512], ps_o[:])
                        else:
                            nc.scalar.copy(o_bf[:, 512:1024], ps_o[:])
                    r0 = c0 + st * 128
                    nc.sync.dma_start(out=out_d[r0:r0 + 128, :], in_=o_bf[:])

            # interleave: batch phases ahead of their chunks
            phase_a(0)
            phase_a(1)
            for c in range(0, CH_PER_B):
                phase_b(c)
            phase_a(2)
            for c in range(CH_PER_B, 2 * CH_PER_B):
                phase_b(c)
            phase_a(3)
            for c in range(2 * CH_PER_B, NCH):
                phase_b(c)

    nc.compile()
    return nc


def _get_nc():
    if "nc" not in _NC_CACHE:
        _NC_CACHE["nc"] = _build()
    return _NC_CACHE["nc"]


def _prep_core_inputs(x, y, latent_tokens, gate, W_down, W_up, core):
    b0 = core * B
    xs = x[b0:b0 + B].reshape(S, D).astype(BF16)
    xT = np.ascontiguousarray(xs.T).reshape(KD, 128, S)
    ys = y[b0:b0 + B].astype(BF16)
    yT = np.ascontiguousarray(ys.transpose(0, 2, 1)).reshape(B, KD, 128, SY)
    ynat = np.ascontiguousarray(ys).reshape(B, JT, 128, D)
    return {"xT": xT, "yT": yT, "ynat": ynat}


def kernel(x, y, latent_tokens, gate, W_down, W_up):
    from concourse import bass_utils

    trace = bool(int(os.environ.get("KERNEL_TRACE", "0")))
    if trace:
        _install_axon_ntff_hook()
        bass_utils.upload_artifacts = lambda tmpdir: tmpdir

    nc = _get_nc()

    shared = {
        "latT": np.ascontiguousarray(latent_tokens.T.astype(BF16)).reshape(KD, 128, T),
        "latent": latent_tokens.astype(np.float32),
        "wdT": np.ascontiguousarray(W_down.T.astype(BF16)).reshape(KD, 128, E),
        "wuT": np.ascontiguousarray(W_up.T.astype(BF16)),
        "gate128": np.full((128, 1), np.float32(gate.reshape(-1)[0]), np.float32),
        "id2": np.eye(T, dtype=BF16),
    }
    in_maps = []
    for core in range(N_CORES):
        m = dict(shared)
        m.update(_prep_core_inputs(x, y, latent_tokens, gate, W_down, W_up, core))
        in_maps.append(m)

    res = bass_utils.run_bass_kernel_spmd(
        nc, in_maps, core_ids=list(range(N_CORES)), trace=trace
    )
    global LAST_RESULT
    LAST_RESULT = res

    out = np.empty((B_GLOBAL, SX, O), np.float32)
    for core in range(N_CORES):
        out[core * B:(core + 1) * B] = (
            res.results[core]["out"].astype(np.float32).reshape(B, SX, O)
        )
    return out


# revision 8
# speedup vs baseline: 1.3807x; 1.3807x over previous
"""Trainium2 Bass kernel for the Adapter module (nn_Adapter_63436666962301).

Data-parallel over batch: B=32 split as 4 batches per NeuronCore x 8 cores.
Math per batch (reference):
  att_y2t = softmax(latent @ y^T, axis=j)           [T, Sy]
  tokens  = latent + att_y2t @ y                    [T, D]
  att_t2x = softmax(x @ tokens^T, axis=t)           [Sx, T]
  x_new   = x + gate * (att_t2x @ tokens)
  out     = relu(x_new @ W_down^T) @ W_up^T

On-chip the gated attention is folded into the down projection:
  z_preT[e, s] = sum_d WdT[d, e]^T xT[d, s] + (gate * tokens@Wd^T)^T[e, t] attT[t, s]
(exact by distributivity), so the big x tensor streams through the
TensorEngine only twice (logit-diff pass + down-proj).  The T=2 softmax over
tokens is sigmoid(l0 - l1): the logit difference is accumulated directly by
matmul against precomputed token-difference columns [t0-t1, t1-t0].

Compute dtype: bf16 operands, fp32 PSUM accumulation (rel err ~3e-3).
"""

import os
import sys
import types

import numpy as np
import ml_dtypes

BF16 = ml_dtypes.bfloat16

# ---- problem constants (hardcoded; kernel.py must be self-contained) ----
N_CORES = 8
B_GLOBAL = 32
B = B_GLOBAL // N_CORES  # 4 batches per core
SX = 2048
SY = 512
D = 1024
T = 2
E = 128   # bottleneck dim (D // 8)
O = 1024  # output dim
S = B * SX              # 8192 rows of x per core
CH = 512                # s-chunk width
NCH = S // CH           # 16 chunks
KD = D // 128           # 8 contraction tiles
CH_PER_B = SX // CH     # 4 chunks per batch
JT = SY // 128          # 4 j-tiles of y


def _install_axon_ntff_hook():
    """Register the NTFF profiling hook that this image's antenv lacks."""
    try:
        from antenv.axon_hooks import get_axon_ntff_profile_hook  # noqa: F401
        return
    except ImportError:
        pass
    try:
        import antenv
        from trn_agent_boot.trn_boot import _ntff_profile_via_ctypes
        hook = _ntff_profile_via_ctypes("/opt/axon/libaxon_pjrt.so")
    except Exception:
        return
    mod = types.ModuleType("antenv.axon_hooks")
    mod._hook = hook
    mod.get_axon_ntff_profile_hook = lambda: mod._hook

    def _set(h):
        mod._hook = h

    mod.set_axon_ntff_profile_hook = _set
    sys.modules["antenv.axon_hooks"] = mod
    antenv.axon_hooks = mod


_NC_CACHE = {}
LAST_RESULT = None  # test.py reads exec_time_ns from here


def _build():
    import concourse.bass as bass
    import concourse.tile as tile
    from concourse import bacc, mybir

    f32 = mybir.dt.float32
    bf16 = mybir.dt.bfloat16

    nc = bacc.Bacc("TRN2", target_bir_lowering=False, debug=False)

    # ---- DRAM parameters (per-core shard shapes) ----
    xT_d = nc.dram_tensor("xT", [KD, 128, S], bf16, kind="ExternalInput").ap()
    yT_d = nc.dram_tensor("yT", [B, KD, 128, SY], bf16, kind="ExternalInput").ap()
    yn_d = nc.dram_tensor("ynat", [B, JT, 128, D], bf16, kind="ExternalInput").ap()
    latT_d = nc.dram_tensor("latT", [KD, 128, T], bf16, kind="ExternalInput").ap()
    lat_d = nc.dram_tensor("latent", [T, D], f32, kind="ExternalInput").ap()
    wdT_d = nc.dram_tensor("wdT", [KD, 128, E], bf16, kind="ExternalInput").ap()
    wuT_d = nc.dram_tensor("wuT", [E, O], bf16, kind="ExternalInput").ap()
    gate_d = nc.dram_tensor("gate128", [128, 1], f32, kind="ExternalInput").ap()
    id2_d = nc.dram_tensor("id2", [T, T], bf16, kind="ExternalInput").ap()
    out_d = nc.dram_tensor("out", [S, O], bf16, kind="ExternalOutput").ap()

    with tile.TileContext(nc) as tc:
        with (
            tc.tile_pool(name="const", bufs=1) as const,
            tc.tile_pool(name="ypool", bufs=2) as ypool,
            tc.tile_pool(name="xpool", bufs=4) as xpool,
            tc.tile_pool(name="work", bufs=2) as work,
            tc.tile_pool(name="tokw", bufs=1) as tokw,
            tc.tile_pool(name="psum", bufs=1, space="PSUM") as psum,
        ):
            # ---- constants ----
            wdT_sb = const.tile([128, KD, E], bf16)
            nc.sync.dma_start(out=wdT_sb[:], in_=wdT_d.rearrange("k p e -> p k e"))
            wuT_sb = const.tile([E, O], bf16)
            nc.sync.dma_start(out=wuT_sb[:], in_=wuT_d[:])
            latT_sb = const.tile([128, KD, T], bf16)
            nc.sync.dma_start(out=latT_sb[:], in_=latT_d.rearrange("k p t -> p k t"))
            lat_sb = const.tile([T, D], f32)
            nc.sync.dma_start(out=lat_sb[:], in_=lat_d[:])
            gate_sb = const.tile([128, 1], f32)
            nc.sync.dma_start(out=gate_sb[:], in_=gate_d[:])
            id2_sb = const.tile([T, T], bf16)
            nc.sync.dma_start(out=id2_sb[:], in_=id2_d[:])

            # per-batch token state (lives across the batch's 4 chunks)
            tokT_sb = tokw.tile([128, B, KD, T], bf16)   # tokens^T, bf16
            tokDT_sb = tokw.tile([128, B, KD, T], bf16)  # [t0-t1, t1-t0] columns
            gtd_sb = tokw.tile([T, B, E], bf16)          # gate * (tokens @ Wd^T)

            x_tiles = {}

            def load_x(c):
                c0 = c * CH
                x_sb = xpool.tile([128, KD, CH], bf16, tag="xT", name=f"x_sb{c}")
                nc.sync.dma_start(
                    out=x_sb[:], in_=xT_d[:, :, c0:c0 + CH].rearrange("k p s -> p k s")
                )
                x_tiles[c] = x_sb

            def phase_a(b):
                """Per-batch: y2t attention -> tokens -> tokensT, gate*tokens_down."""
                yT_sb = ypool.tile([128, KD, SY], bf16, tag="yT")
                nc.sync.dma_start(out=yT_sb[:], in_=yT_d[b].rearrange("k p j -> p k j"))
                yn_sb = ypool.tile([128, JT, D], bf16, tag="ynat")
                nc.sync.dma_start(out=yn_sb[:], in_=yn_d[b].rearrange("j p d -> p j d"))

                # scores[t, j] = latent @ y^T (contraction over d)
                ps_sc = psum.tile([T, SY], f32, tag="small", bufs=2)
                for kd in range(KD):
                    nc.tensor.matmul(
                        ps_sc[:], latT_sb[:, kd, :], yT_sb[:, kd, :],
                        start=(kd == 0), stop=(kd == KD - 1),
                    )
                # softmax over j (free dim)
                negmx = work.tile([T, 1], f32, tag="small")
                nc.vector.tensor_reduce(
                    negmx[:], ps_sc[:], mybir.AxisListType.X, mybir.AluOpType.max,
                    negate=True,
                )
                e_sb = work.tile([T, SY], f32, tag="esb")
                nc.scalar.activation(
                    e_sb[:], ps_sc[:], mybir.ActivationFunctionType.Exp,
                    bias=negmx[:], scale=1.0,
                )
                ssum = work.tile([T, 1], f32, tag="small")
                nc.vector.tensor_reduce(
                    ssum[:], e_sb[:], mybir.AxisListType.X, mybir.AluOpType.add,
                )
                rinv = work.tile([T, 1], f32, tag="small")
                nc.vector.reciprocal(rinv[:], ssum[:])
                att_bf = work.tile([T, SY], bf16, tag="atty")
                nc.vector.tensor_scalar_mul(att_bf[:], e_sb[:], rinv[:])

                # att^T via batched PE transposes into one PSUM tile, one copy out
                attT_sb = work.tile([128, JT, T], bf16, tag="attT")
                ps_at = psum.tile([128, JT, T], bf16, tag="small", bufs=2)
                for jt in range(JT):
                    nc.tensor.transpose(
                        ps_at[:, jt, :], att_bf[:, jt * 128:(jt + 1) * 128], id2_sb[:]
                    )
                nc.vector.tensor_copy(attT_sb[:], ps_at[:])

                # tokens[t, d] = latent + att @ y  (contraction over j), halves of d
                tok_bf = work.tile([T, D], bf16, tag="tok")
                for dh in range(2):
                    ps_tok = psum.tile([T, 512], f32, tag="small", bufs=2)
                    for jt in range(JT):
                        nc.tensor.matmul(
                            ps_tok[:], attT_sb[:, jt, :],
                            yn_sb[:, jt, dh * 512:(dh + 1) * 512],
                            start=(jt == 0), stop=(jt == JT - 1),
                        )
                    nc.vector.tensor_add(
                        tok_bf[:, dh * 512:(dh + 1) * 512], ps_tok[:],
                        lat_sb[:, dh * 512:(dh + 1) * 512],
                    )

                # tokens^T via batched PE transposes, one copy out
                ps_tt = psum.tile([128, KD, T], bf16, tag="small", bufs=2)
                for kd in range(KD):
                    nc.tensor.transpose(
                        ps_tt[:, kd, :], tok_bf[:, kd * 128:(kd + 1) * 128], id2_sb[:]
                    )
                nc.vector.tensor_copy(tokT_sb[:, b, :, :], ps_tt[:])
                # difference columns for the T=2 softmax-as-sigmoid
                nc.vector.tensor_sub(
                    tokDT_sb[:, b, :, 0:1], tokT_sb[:, b, :, 0:1], tokT_sb[:, b, :, 1:2],
                )
                nc.vector.tensor_sub(
                    tokDT_sb[:, b, :, 1:2], tokT_sb[:, b, :, 1:2], tokT_sb[:, b, :, 0:1],
                )

                # tokens_down[t, e] = tokens @ Wd^T, then scale by gate
                ps_td = psum.tile([T, E], f32, tag="small", bufs=2)
                for kd in range(KD):
                    nc.tensor.matmul(
                        ps_td[:], tokT_sb[:, b, kd, :], wdT_sb[:, kd, :],
                        start=(kd == 0), stop=(kd == KD - 1),
                    )
                nc.vector.tensor_scalar_mul(gtd_sb[:, b, :], ps_td[:], gate_sb[0:T, :])

            def phase_b(c):
                """Per s-chunk of 512 rows: t2x attention + down + up projection."""
                b = c // CH_PER_B
                c0 = c * CH
                x_sb = x_tiles.pop(c)

                # logit diffs [l0-l1; l1-l0] directly (contraction over d)
                ps_dd = psum.tile([T, CH], f32, tag="small", bufs=2)
                for kd in range(KD):
                    nc.tensor.matmul(
                        ps_dd[:], tokDT_sb[:, b, kd, :], x_sb[:, kd, :],
                        start=(kd == 0), stop=(kd == KD - 1),
                    )
                attx_bf = work.tile([T, CH], bf16, tag="attx", bufs=3)
                nc.scalar.activation(
                    attx_bf[:], ps_dd[:], mybir.ActivationFunctionType.Sigmoid,
                )

                # z^T[e, s] = Wd @ x_new^T  (+ gated attention term), then relu
                ps_z = psum.tile([E, CH], f32, tag="z", bufs=2)
                for kd in range(KD):
                    nc.tensor.matmul(
                        ps_z[:], wdT_sb[:, kd, :], x_sb[:, kd, :],
                        start=(kd == 0), stop=False,
                    )
                nc.tensor.matmul(
                    ps_z[:], gtd_sb[:, b, :], attx_bf[:],
                    start=False, stop=True,
                )
                z_bf = work.tile([E, CH], bf16, tag="z_bf", bufs=3)
                nc.scalar.activation(
                    z_bf[:], ps_z[:], mybir.ActivationFunctionType.Relu,
                )

                # out[s, o] = z @ Wu^T  (contraction over e=128, single K tile)
                o_bf = work.tile([128, 4, O], bf16, tag="obf", bufs=2)
                for st in range(4):
                    ps_o = psum.tile([128, O], f32, tag="o", bufs=2)
                    for oh in range(2):
                        nc.tensor.matmul(
                            ps_o[:, oh * 512:(oh + 1) * 512],
                            z_bf[:, st * 128:(st + 1) * 128],
                            wuT_sb[:, oh * 512:(oh + 1) * 512],
                        )
                    if st % 2 == 0:
                        nc.vector.tensor_copy(o_bf[:, st, :], ps_o[:])
                    else:
                        nc.scalar.copy(o_bf[:, st, :], ps_o[:])
                nc.gpsimd.dma_start(
                    out=out_d[c0:c0 + CH, :].rearrange("(st p) o -> p st o", p=128),
                    in_=o_bf[:],
                )

            # pipelined emission: prefetch x, run batch phases ahead of chunks
            load_x(0)
            load_x(1)
            phase_a(0)
            load_x(2)
            phase_a(1)
            for c in range(0, CH_PER_B):
                load_x(c + 3)
                phase_b(c)
            phase_a(2)
            for c in range(CH_PER_B, 2 * CH_PER_B):
                load_x(c + 3)
                phase_b(c)
            phase_a(3)
            for c in range(2 * CH_PER_B, NCH):
                if c + 3 < NCH:
                    load_x(c + 3)
                phase_b(c)

    nc.compile()
    return nc


def _get_nc():
    if "nc" not in _NC_CACHE:
        _NC_CACHE["nc"] = _build()
    return _NC_CACHE["nc"]


def _prep_core_inputs(x, y, latent_tokens, gate, W_down, W_up, core):
    b0 = core * B
    xs = x[b0:b0 + B].reshape(S, D).astype(BF16)
    xT = np.ascontiguousarray(xs.T).reshape(KD, 128, S)
    ys = y[b0:b0 + B].astype(BF16)
    yT = np.ascontiguousarray(ys.transpose(0, 2, 1)).reshape(B, KD, 128, SY)
    ynat = np.ascontiguousarray(ys).reshape(B, JT, 128, D)
    return {"xT": xT, "yT": yT, "ynat": ynat}


def kernel(x, y, latent_tokens, gate, W_down, W_up):
    from concourse import bass_utils

    trace = bool(int(os.environ.get("KERNEL_TRACE", "0")))
    if trace:
        _install_axon_ntff_hook()
        bass_utils.upload_artifacts = lambda tmpdir: tmpdir

    nc = _get_nc()

    shared = {
        "latT": np.ascontiguousarray(latent_tokens.T.astype(BF16)).reshape(KD, 128, T),
        "latent": latent_tokens.astype(np.float32),
        "wdT": np.ascontiguousarray(W_down.T.astype(BF16)).reshape(KD, 128, E),
        "wuT": np.ascontiguousarray(W_up.T.astype(BF16)),
        "gate128": np.full((128, 1), np.float32(gate.reshape(-1)[0]), np.float32),
        "id2": np.eye(T, dtype=BF16),
    }
    in_maps = []
    for core in range(N_CORES):
        m = dict(shared)
        m.update(_prep_core_inputs(x, y, latent_tokens, gate, W_down, W_up, core))
        in_maps.append(m)

    res = bass_utils.run_bass_kernel_spmd(
        nc, in_maps, core_ids=list(range(N_CORES)), trace=trace
    )
    global LAST_RESULT
    LAST_RESULT = res

    out = np.empty((B_GLOBAL, SX, O), np.float32)
    for core in range(N_CORES):
        out[core * B:(core + 1) * B] = (
            res.results[core]["out"].astype(np.float32).reshape(B, SX, O)
        )
    return out


# revision 9
# speedup vs baseline: 1.4513x; 1.0511x over previous
"""Trainium2 Bass kernel for the Adapter module (nn_Adapter_63436666962301).

Data-parallel over batch: B=32 split as 4 batches per NeuronCore x 8 cores.
Math per batch (reference):
  att_y2t = softmax(latent @ y^T, axis=j)           [T, Sy]
  tokens  = latent + att_y2t @ y                    [T, D]
  att_t2x = softmax(x @ tokens^T, axis=t)           [Sx, T]
  x_new   = x + gate * (att_t2x @ tokens)
  out     = relu(x_new @ W_down^T) @ W_up^T

On-chip the gated attention is folded into the down projection:
  z_preT[e, s] = sum_d WdT[d, e]^T xT[d, s] + (gate * tokens@Wd^T)^T[e, t] attT[t, s]
(exact by distributivity), so the big x tensor streams through the
TensorEngine only twice (logit-diff pass + down-proj).  The T=2 softmax over
tokens is sigmoid(l0 - l1): the logit difference is accumulated directly by
matmul against precomputed token-difference columns [t0-t1, t1-t0].

Compute dtype: bf16 operands, fp32 PSUM accumulation (rel err ~3e-3).
"""

import os
import sys
import types

import numpy as np
import ml_dtypes

BF16 = ml_dtypes.bfloat16

# ---- problem constants (hardcoded; kernel.py must be self-contained) ----
N_CORES = 8
B_GLOBAL = 32
B = B_GLOBAL // N_CORES  # 4 batches per core
SX = 2048
SY = 512
D = 1024
T = 2
E = 128   # bottleneck dim (D // 8)
O = 1024  # output dim
S = B * SX              # 8192 rows of x per core
CH = 512                # s-chunk width
NCH = S // CH           # 16 chunks
KD = D // 128           # 8 contraction tiles
CH_PER_B = SX // CH     # 4 chunks per batch
JT = SY // 128          # 4 j-tiles of y


def _install_axon_ntff_hook():
    """Register the NTFF profiling hook that this image's antenv lacks."""
    try:
        from antenv.axon_hooks import get_axon_ntff_profile_hook  # noqa: F401
        return
    except ImportError:
        pass
    try:
        import antenv
        from trn_agent_boot.trn_boot import _ntff_profile_via_ctypes
        hook = _ntff_profile_via_ctypes("/opt/axon/libaxon_pjrt.so")
    except Exception:
        return
    mod = types.ModuleType("antenv.axon_hooks")
    mod._hook = hook
    mod.get_axon_ntff_profile_hook = lambda: mod._hook

    def _set(h):
        mod._hook = h

    mod.set_axon_ntff_profile_hook = _set
    sys.modules["antenv.axon_hooks"] = mod
    antenv.axon_hooks = mod


_NC_CACHE = {}
LAST_RESULT = None  # test.py reads exec_time_ns from here


def _build():
    import concourse.bass as bass
    import concourse.tile as tile
    from concourse import bacc, mybir

    f32 = mybir.dt.float32
    bf16 = mybir.dt.bfloat16

    nc = bacc.Bacc("TRN2", target_bir_lowering=False, debug=False)

    # ---- DRAM parameters (per-core shard shapes) ----
    xT_d = nc.dram_tensor("xT", [KD, 128, S], bf16, kind="ExternalInput").ap()
    yT_d = nc.dram_tensor("yT", [B, KD, 128, SY], bf16, kind="ExternalInput").ap()
    yn_d = nc.dram_tensor("ynat", [B, JT, 128, D], bf16, kind="ExternalInput").ap()
    latT_d = nc.dram_tensor("latT", [KD, 128, T], bf16, kind="ExternalInput").ap()
    lat_d = nc.dram_tensor("latent", [T, D], f32, kind="ExternalInput").ap()
    wdT_d = nc.dram_tensor("wdT", [KD, 128, E], bf16, kind="ExternalInput").ap()
    wuT_d = nc.dram_tensor("wuT", [E, O], bf16, kind="ExternalInput").ap()
    gate_d = nc.dram_tensor("gate128", [128, 1], f32, kind="ExternalInput").ap()
    id2_d = nc.dram_tensor("id2", [T, T], bf16, kind="ExternalInput").ap()
    out_d = nc.dram_tensor("out", [S, O], bf16, kind="ExternalOutput").ap()

    with tile.TileContext(nc) as tc:
        with (
            tc.tile_pool(name="const", bufs=1) as const,
            tc.tile_pool(name="ypool", bufs=2) as ypool,
            tc.tile_pool(name="xpool", bufs=4) as xpool,
            tc.tile_pool(name="work", bufs=2) as work,
            tc.tile_pool(name="tokw", bufs=1) as tokw,
            tc.tile_pool(name="psum", bufs=1, space="PSUM") as psum,
        ):
            # ---- constants ----
            wdT_sb = const.tile([128, KD, E], bf16)
            nc.sync.dma_start(out=wdT_sb[:], in_=wdT_d.rearrange("k p e -> p k e"))
            wuT_sb = const.tile([E, O], bf16)
            nc.sync.dma_start(out=wuT_sb[:], in_=wuT_d[:])
            latT_sb = const.tile([128, KD, T], bf16)
            nc.sync.dma_start(out=latT_sb[:], in_=latT_d.rearrange("k p t -> p k t"))
            lat_sb = const.tile([T, D], f32)
            nc.sync.dma_start(out=lat_sb[:], in_=lat_d[:])
            gate_sb = const.tile([128, 1], f32)
            nc.sync.dma_start(out=gate_sb[:], in_=gate_d[:])
            id2_sb = const.tile([T, T], bf16)
            nc.sync.dma_start(out=id2_sb[:], in_=id2_d[:])

            # per-batch token state (lives across the batch's 4 chunks)
            tokT_sb = tokw.tile([128, B, KD, T], bf16)   # tokens^T, bf16
            tokDT_sb = tokw.tile([128, B, KD, T], bf16)  # [t0-t1, t1-t0] columns
            gtd_sb = tokw.tile([T, B, E], bf16)          # gate * (tokens @ Wd^T)

            x_tiles = {}

            def load_x(c):
                c0 = c * CH
                x_sb = xpool.tile([128, KD, CH], bf16, tag="xT", name=f"x_sb{c}")
                nc.sync.dma_start(
                    out=x_sb[:], in_=xT_d[:, :, c0:c0 + CH].rearrange("k p s -> p k s")
                )
                x_tiles[c] = x_sb

            def phase_a(b):
                """Per-batch: y2t attention -> tokens -> tokensT, gate*tokens_down."""
                yT_sb = ypool.tile([128, KD, SY], bf16, tag="yT")
                nc.sync.dma_start(out=yT_sb[:], in_=yT_d[b].rearrange("k p j -> p k j"))
                yn_sb = ypool.tile([128, JT, D], bf16, tag="ynat")
                nc.sync.dma_start(out=yn_sb[:], in_=yn_d[b].rearrange("j p d -> p j d"))

                # scores[t, j] = latent @ y^T (contraction over d)
                ps_sc = psum.tile([T, SY], f32, tag="small", bufs=1)
                for kd in range(KD):
                    nc.tensor.matmul(
                        ps_sc[:], latT_sb[:, kd, :], yT_sb[:, kd, :],
                        start=(kd == 0), stop=(kd == KD - 1),
                    )
                # softmax over j (free dim); normalization folded into tokens
                negmx = work.tile([T, 1], f32, tag="small")
                nc.vector.tensor_reduce(
                    negmx[:], ps_sc[:], mybir.AxisListType.X, mybir.AluOpType.max,
                    negate=True,
                )
                e_bf = work.tile([T, SY], bf16, tag="atty")
                nc.scalar.activation(
                    e_bf[:], ps_sc[:], mybir.ActivationFunctionType.Exp,
                    bias=negmx[:], scale=1.0,
                )
                ssum = work.tile([T, 1], f32, tag="small")
                nc.vector.tensor_reduce(
                    ssum[:], e_bf[:], mybir.AxisListType.X, mybir.AluOpType.add,
                )
                rinv = work.tile([T, 1], f32, tag="small")
                nc.vector.reciprocal(rinv[:], ssum[:])

                # e^T via batched PE transposes into one PSUM tile, one copy out
                eT_sb = work.tile([128, JT, T], bf16, tag="attT")
                ps_at = psum.tile([128, JT, T], bf16, tag="small", bufs=1)
                for jt in range(JT):
                    nc.tensor.transpose(
                        ps_at[:, jt, :], e_bf[:, jt * 128:(jt + 1) * 128], id2_sb[:]
                    )
                nc.vector.tensor_copy(eT_sb[:], ps_at[:])

                # tokens[t, d] = latent + rinv * (e @ y), halves of d
                tok_bf = work.tile([T, D], bf16, tag="tok")
                for dh in range(2):
                    ps_tok = psum.tile([T, 512], f32, tag="small", bufs=1)
                    for jt in range(JT):
                        nc.tensor.matmul(
                            ps_tok[:], eT_sb[:, jt, :],
                            yn_sb[:, jt, dh * 512:(dh + 1) * 512],
                            start=(jt == 0), stop=(jt == JT - 1),
                        )
                    nc.vector.scalar_tensor_tensor(
                        tok_bf[:, dh * 512:(dh + 1) * 512], ps_tok[:], rinv[:],
                        lat_sb[:, dh * 512:(dh + 1) * 512],
                        mybir.AluOpType.mult, mybir.AluOpType.add,
                    )

                # tokens^T via batched PE transposes, one copy out
                ps_tt = psum.tile([128, KD, T], bf16, tag="small", bufs=1)
                for kd in range(KD):
                    nc.tensor.transpose(
                        ps_tt[:, kd, :], tok_bf[:, kd * 128:(kd + 1) * 128], id2_sb[:]
                    )
                nc.vector.tensor_copy(tokT_sb[:, b, :, :], ps_tt[:])
                # difference columns for the T=2 softmax-as-sigmoid
                nc.vector.tensor_sub(
                    tokDT_sb[:, b, :, 0:1], tokT_sb[:, b, :, 0:1], tokT_sb[:, b, :, 1:2],
                )
                nc.vector.tensor_sub(
                    tokDT_sb[:, b, :, 1:2], tokT_sb[:, b, :, 1:2], tokT_sb[:, b, :, 0:1],
                )

                # tokens_down[t, e] = tokens @ Wd^T, then scale by gate
                ps_td = psum.tile([T, E], f32, tag="small", bufs=1)
                for kd in range(KD):
                    nc.tensor.matmul(
                        ps_td[:], tokT_sb[:, b, kd, :], wdT_sb[:, kd, :],
                        start=(kd == 0), stop=(kd == KD - 1),
                    )
                nc.vector.tensor_scalar_mul(gtd_sb[:, b, :], ps_td[:], gate_sb[0:T, :])

            z_state = {}

            def phase_b_front(c):
                """Per s-chunk: down-proj accumulation + logit-diff pass."""
                b = c // CH_PER_B
                x_sb = x_tiles.pop(c)

                # z^T[e, s] = Wd @ x^T accumulation (group left open for gate term)
                ps_z = psum.tile([E, CH], f32, tag="z", bufs=2)
                for kd in range(KD):
                    nc.tensor.matmul(
                        ps_z[:], wdT_sb[:, kd, :], x_sb[:, kd, :],
                        start=(kd == 0), stop=False,
                    )
                # logit diffs [l0-l1; l1-l0] directly (contraction over d)
                ps_dd = psum.tile([T, CH], f32, tag="dd", bufs=2)
                for kd in range(KD):
                    nc.tensor.matmul(
                        ps_dd[:], tokDT_sb[:, b, kd, :], x_sb[:, kd, :],
                        start=(kd == 0), stop=(kd == KD - 1),
                    )
                attx_bf = work.tile([T, CH], bf16, tag="attx", bufs=3)
                nc.scalar.activation(
                    attx_bf[:], ps_dd[:], mybir.ActivationFunctionType.Sigmoid,
                )
                z_state[c] = (ps_z, attx_bf, b)

            def phase_b_mid(c):
                """Gated attention term into the open z accumulation, then relu."""
                ps_z, attx_bf, b = z_state.pop(c)
                nc.tensor.matmul(
                    ps_z[:], gtd_sb[:, b, :], attx_bf[:],
                    start=False, stop=True,
                )
                z_bf = work.tile([E, CH], bf16, tag="z_bf", bufs=3)
                nc.scalar.activation(
                    z_bf[:], ps_z[:], mybir.ActivationFunctionType.Relu,
                )
                return z_bf

            z_tiles = {}

            def phase_b_back(c):
                """Up-projection of a finished chunk + store."""
                c0 = c * CH
                z_bf = z_tiles.pop(c)
                o_bf = work.tile([128, 4, O], bf16, tag="obf", bufs=2)
                for st in range(4):
                    ps_o = psum.tile([128, 512], f32, tag="o", bufs=3)
                    nc.tensor.matmul(
                        ps_o[:], z_bf[:, st * 128:(st + 1) * 128],
                        wuT_sb[:, 0:512],
                    )
                    ps_o2 = psum.tile([128, 512], f32, tag="o", bufs=3)
                    nc.tensor.matmul(
                        ps_o2[:], z_bf[:, st * 128:(st + 1) * 128],
                        wuT_sb[:, 512:1024],
                    )
                    if st % 2 == 0:
                        nc.vector.tensor_copy(o_bf[:, st, 0:512], ps_o[:])
                        nc.scalar.copy(o_bf[:, st, 512:1024], ps_o2[:])
                    else:
                        nc.scalar.copy(o_bf[:, st, 0:512], ps_o[:])
                        nc.vector.tensor_copy(o_bf[:, st, 512:1024], ps_o2[:])
                nc.gpsimd.dma_start(
                    out=out_d[c0:c0 + CH, :].rearrange("(st p) o -> p st o", p=128),
                    in_=o_bf[:],
                )

            # pipelined emission: A-phases and up-proj woven between chunk fronts
            phase_a(0)
            load_x(0)
            load_x(1)
            phase_a(1)
            load_x(2)
            for c in range(NCH):
                if c + 3 < NCH:
                    load_x(c + 3)
                phase_b_front(c)
                if c - 1 >= 0:
                    phase_b_back(c - 1)
                z_tiles[c] = phase_b_mid(c)
                if c == 5:
                    phase_a(2)
                if c == 9:
                    phase_a(3)
            phase_b_back(NCH - 1)

    nc.compile()
    return nc


def _get_nc():
    if "nc" not in _NC_CACHE:
        _NC_CACHE["nc"] = _build()
    return _NC_CACHE["nc"]


def _prep_core_inputs(x, y, latent_tokens, gate, W_down, W_up, core):
    b0 = core * B
    xs = x[b0:b0 + B].reshape(S, D).astype(BF16)
    xT = np.ascontiguousarray(xs.T).reshape(KD, 128, S)
    ys = y[b0:b0 + B].astype(BF16)
    yT = np.ascontiguousarray(ys.transpose(0, 2, 1)).reshape(B, KD, 128, SY)
    ynat = np.ascontiguousarray(ys).reshape(B, JT, 128, D)
    return {"xT": xT, "yT": yT, "ynat": ynat}


def kernel(x, y, latent_tokens, gate, W_down, W_up):
    from concourse import bass_utils

    trace = bool(int(os.environ.get("KERNEL_TRACE", "0")))
    if trace:
        _install_axon_ntff_hook()
        bass_utils.upload_artifacts = lambda tmpdir: tmpdir

    nc = _get_nc()

    shared = {
        "latT": np.ascontiguousarray(latent_tokens.T.astype(BF16)).reshape(KD, 128, T),
        "latent": latent_tokens.astype(np.float32),
        "wdT": np.ascontiguousarray(W_down.T.astype(BF16)).reshape(KD, 128, E),
        "wuT": np.ascontiguousarray(W_up.T.astype(BF16)),
        "gate128": np.full((128, 1), np.float32(gate.reshape(-1)[0]), np.float32),
        "id2": np.eye(T, dtype=BF16),
    }
    in_maps = []
    for core in range(N_CORES):
        m = dict(shared)
        m.update(_prep_core_inputs(x, y, latent_tokens, gate, W_down, W_up, core))
        in_maps.append(m)

    res = bass_utils.run_bass_kernel_spmd(
        nc, in_maps, core_ids=list(range(N_CORES)), trace=trace
    )
    global LAST_RESULT
    LAST_RESULT = res

    out = np.empty((B_GLOBAL, SX, O), np.float32)
    for core in range(N_CORES):
        out[core * B:(core + 1) * B] = (
            res.results[core]["out"].astype(np.float32).reshape(B, SX, O)
        )
    return out
